# revision 10
# baseline (speedup 1.0000x reference)
"""GATv2 (2-layer, heads=1) on 8 trn2 NeuronCores.

Strategy v2 (dst-sharded, per-side class-scheduled):
  - Nodes partitioned across 8 cores by dst id (6250 each). Edges (with self
    loops) grouped by (dst, side) where side A = src on devices 0-3, side B =
    src on devices 4-7 (keeps gather indices < 32768 for int16 dma_gather).
  - Pair-classes (KA, KB) = 4-padded side-degrees fix the node ordering, but
    each side is tiled INDEPENDENTLY: side s of a class uses m_s = 128//K_s
    dst-groups per tile (te_s = m_s*K_s slots), so both sides pack ~94% of
    the 128 gather slots (v1 paired tiles wasted the smaller side).
  - Every gather descriptor fetches a real row; padding slots fetch row 0
    whose p-column is -100 so exp() kills them.
  - No layer-1 AllGather: every core computes the FULL layer-1 node table
    locally from the replicated (slot-ordered) x, in bf16.
  - Layer-1 logit:  e = att.leaky_relu(xl[src]+xr[dst])
                    = 0.8*((p_j+q_i)/4 + relu-sum over sign-sorted, |att|-
                      scaled feature columns), with p=att.xl/4, q=att.xr/4.
    Softmax max-subtraction is skipped (shift-invariant; exponents small).
  - h (layer-1 output) is never materialized: layer 2 only needs
    xl2 = h@Wl2, xr2 = h@Wr2, linear in the layer-1 aggregates of
    y = xl@Wl2, z = xl@Wr2 (2 cols each).
  - Aggregation: s = exp(e) per slot; values scaled by s on DVE, then ONE
    batched matmul per (batch, side) with a constant per-class one-hot lhsT
    accumulates [m_s, NV] per tile into that side's PSUM window. Windows
    flush (PSUM -> SBUF copy -> DMA) into per-side DRAM accumulators X_s in
    node-slot order; a vectorized final pass adds the sides, normalizes and
    emits the layer-2 tables / output.
  - Layer-2 table (16B/row used) is shared via ONE strided AllGather that
    moves only the used bytes; rows keep 256B stride for dma_gather.
  - Both passes use the same schedule, table layout and index arrays.
"""

import os
import numpy as np
import ml_dtypes

N = 50000
E = 800000
FIN = 128
HID = 64
FOUT = 2
NCORE = 8
NPD = N // NCORE  # 6250 dst nodes per device
NHALF = N // 2    # src < NHALF -> side A

SBW = 48   # tiles (chunks) per gather super-call
BAT = 7    # tiles per matmul batch (layer 1; 65 f32 cols cap PSUM at 7)
BAT2 = 48  # tiles per matmul batch (layer 2; rows are tiny)
PSW = 97   # psum window capacity in tiles (97*5*4B = 1940B <= 2KB)
DCH = 16   # dense chunks per DMA write batch
FCH = 16   # final-pass chunks per iteration

ROW1_U16 = 128  # L1 table row: 65 bf16 | pad | 4 f32 (yz) | pad (256B)
ROW1_F32O = 33  # f32 view offset of [y0 y1 z0 z1]
ROW2_U16 = 128  # L2 table row: 3 bf16 | pad | 2 f32 | pad (256B)
ROW2_F32O = 2   # f32 view offset of [xl2_0 xl2_1]
ROW2_USED = 8   # u16 cols actually used (16B) -> strided AllGather width
XR1_U16 = 66    # xr row: 65 bf16 | pad
XR2_U16 = 4     # xr2 row: 3 bf16 | pad
PCOL_NEG = -100.0  # p-col of zero rows: exp(0.8*(x - 100)) ~ 0


def _ceil(a, b):
    return -(-a // b)


def _class_of(deg):
    # pad side-degree to a multiple of 4 (0 stays 0)
    return np.where(deg > 0, np.maximum(np.ceil(deg / 4).astype(np.int64) * 4, 4), 0)


def build_schedule(edge_index):
    """Host-side scheduling. Pure numpy. Returns meta dict.

    Device assignment: side A/B of an edge is fixed by src id (< NHALF).
    Within each id-half, nodes are assigned to that half's 4 devices
    round-robin per pair-class, so per-device class counts are balanced
    (tiles = max over devices is tight)."""
    src = np.asarray(edge_index[0], dtype=np.int64)
    dst = np.asarray(edge_index[1], dtype=np.int64)
    loops = np.arange(N, dtype=np.int64)
    src = np.concatenate([src, loops])
    dst = np.concatenate([dst, loops])

    sideB = (src >= NHALF).astype(np.int64)  # 0 = A, 1 = B

    # per-(node, side) degree (global, assignment-independent)
    degA = np.bincount(dst[sideB == 0], minlength=N)
    degB = np.bincount(dst[sideB == 1], minlength=N)
    # Reserve a self-loop slot on BOTH sides before class quantization: the
    # self-loop only lands on the node's own half, which would otherwise
    # shift ~27% of nodes across the 4-multiple class boundary differently
    # per half -- and SPMD tiling pays the max over devices.
    inB = (np.arange(N) >= NHALF).astype(np.int64)
    KA_n = _class_of(degA + inB * (degA > 0))  # [N]
    KB_n = _class_of(degB + (1 - inB) * (degB > 0))
    assert max(KA_n.max(), KB_n.max()) <= 124

    pair_n = KA_n * 1000 + KB_n
    pairs = np.unique(pair_n)

    # class-balanced device assignment within each id-half
    dev_of = np.empty(N, dtype=np.int64)
    for h in range(2):
        base = h * NHALF
        for pair in pairs:
            ids = np.where(pair_n[base:base + NHALF] == pair)[0] + base
            dev_of[ids] = h * 4 + np.arange(len(ids)) % 4

    # pair-class list with per-side tiling; tiles from max over devices
    classes = []
    for pair in pairs:
        KA, KB = int(pair // 1000), int(pair % 1000)
        sel = pair_n == pair
        n_k = np.bincount(dev_of[sel], minlength=NCORE)
        nmax = int(n_k.max())
        if nmax == 0:
            continue
        mA = 128 // KA if KA else 0
        mB = 128 // KB if KB else 0
        tA = _ceil(nmax, mA) if KA else 0
        tB = _ceil(nmax, mB) if KB else 0
        span = max(tA * mA if KA else 0, tB * mB if KB else 0)
        classes.append(dict(KA=KA, KB=KB, mA=mA, mB=mB,
                            teA=mA * KA, teB=mB * KB,
                            tilesA=tA, tilesB=tB, span=span))

    # slot layout: slot 0 reserved (zero row); then per class, node-linear
    ns = 1
    for c in classes:
        c["slot0"] = ns
        ns += c["span"]
    NS = ns
    NSP = -(-NS // 128) * 128
    assert 4 * NSP <= 32768, NSP  # int16 gather index limit

    # per-device slot assignment (global node ids)
    slot_glob = np.full(N, -1, dtype=np.int64)   # node -> slot on its device
    node_at = np.full((NCORE, NSP), -1, dtype=np.int64)  # (dev, slot) -> node
    for c in classes:
        pair = c["KA"] * 1000 + c["KB"]
        ids = np.where(pair_n == pair)[0]
        for d in range(NCORE):
            idd = ids[dev_of[ids] == d]
            s = c["slot0"] + np.arange(len(idd))
            slot_glob[idd] = s
            node_at[d, s] = idd

    # table row of a node: dev*NSP + slot (same for L1 table and L2 table)
    halfrows = (NCORE // 2) * NSP
    grow = dev_of[src] * NSP + slot_glob[src]

    # blocks: per class, per side, chunks of <= SBW tiles
    blocks = []
    for c in classes:
        for side in (0, 1):
            tiles = c["tilesA"] if side == 0 else c["tilesB"]
            if tiles == 0:
                continue
            t0 = 0
            while t0 < tiles:
                nch = min(SBW, tiles - t0)
                blocks.append((c, side, t0, nch))
                t0 += nch

    # edge -> (class, side, tile, row-in-tile)
    eslot = dev_of[dst] * NS + slot_glob[dst]
    okey = eslot * 2 + sideB
    order = np.argsort(okey, kind="stable")
    es = eslot[order]
    sb = sideB[order]
    gr = grow[order]
    uniq, start_idx, counts = np.unique(okey[order], return_index=True,
                                        return_counts=True)
    pos = np.arange(len(es)) - np.repeat(start_idx, counts)
    e_dev = es // NS
    e_slot = es % NS

    cls_of_slot = np.zeros(NS, dtype=np.int64)
    for ci, c in enumerate(classes):
        cls_of_slot[c["slot0"]: c["slot0"] + c["span"]] = ci
    cKA = np.array([c["KA"] for c in classes])
    cKB = np.array([c["KB"] for c in classes])
    cmA = np.array([max(c["mA"], 1) for c in classes])
    cmB = np.array([max(c["mB"], 1) for c in classes])
    cs0 = np.array([c["slot0"] for c in classes])
    ci = cls_of_slot[e_slot]
    rel = e_slot - cs0[ci]
    cm = np.where(sb == 0, cmA[ci], cmB[ci])
    cK = np.where(sb == 0, cKA[ci], cKB[ci])
    tt = rel // cm
    gg = rel % cm
    rr = gg * cK + pos
    assert (pos < cK).all()

    # per-block index arrays (wrapped int16), sides separate
    idx_parts = [[], []]  # [A parts, B parts]
    for c, side, t0, nch in blocks:
        cidx = classes.index(c)
        n_tot = 128 * nch
        val = np.zeros((NCORE, n_tot), dtype=np.int64)
        msk = (ci == cidx) & (sb == side) & (tt >= t0) & (tt < t0 + nch)
        n_e = (tt[msk] - t0) * 128 + rr[msk]
        rowv = gr[msk]
        if side == 1:
            rowv = rowv - halfrows
        if len(rowv):
            assert rowv.min() >= 0 and rowv.max() < halfrows
        val[e_dev[msk], n_e] = rowv
        # wrap int16: n -> [n%16, n//16], replicated over 8 core-groups
        w = val.reshape(NCORE, n_tot // 16, 16).transpose(0, 2, 1)
        idx_parts[side].append(np.tile(w, (1, 8, 1)).astype(np.int16))
    idxA = np.concatenate(idx_parts[0], axis=2)
    idxB = np.concatenate(idx_parts[1], axis=2)

    # x rows in slot order, per device (node_at holds global node ids)
    def slot_x(d, x):
        xs = np.zeros((NSP, FIN), dtype=np.float32)
        valid = node_at[d] >= 0
        xs[valid] = x[node_at[d][valid]]
        return xs

    meta = dict(classes=classes, blocks=blocks, NS=NS, NSP=NSP,
                node_at=node_at, idxA=idxA, idxB=idxB, slot_x=slot_x)
    return meta


def prep_weights(Wl1, Wr1, att1, b1, Wl2, Wr2, att2, b2):
    """Host weight transforms. Returns dict of constant arrays."""
    bf = ml_dtypes.bfloat16
    perm1 = np.argsort(att1 < 0, kind="stable")
    a1p = np.abs(att1[perm1])
    npos1 = int((att1 >= 0).sum())
    wtab_att = np.zeros((FIN, 65), dtype=np.float32)
    wtab_att[:, :64] = Wl1[:, perm1] * a1p[None, :]
    wtab_att[:, 64] = 0.25 * (Wl1 @ att1)
    w_yz = np.concatenate([Wl1 @ Wl2, Wl1 @ Wr2], axis=1).astype(np.float32)
    wxr = np.zeros((FIN, 65), dtype=np.float32)
    wxr[:, :64] = Wr1[:, perm1] * a1p[None, :]
    wxr[:, 64] = 0.25 * (Wr1 @ att1)
    # fused dense rhs: [att 65 | yz 4] bf16
    wall = np.concatenate([wtab_att, w_yz], axis=1).astype(bf)

    cy = b1 @ Wl2  # [2]
    cz = b1 @ Wr2
    perm2 = np.argsort(att2 < 0, kind="stable")
    a2p = np.abs(att2[perm2])
    npos2 = int((att2 >= 0).sum())
    return dict(
        wall=wall, wxr=wxr.astype(bf), npos1=npos1,
        cy=cy, cz=cz, perm2=perm2, a2p=a2p, npos2=npos2,
        att2=att2, b2=b2,
    )


def make_oh_consts(classes):
    """Constant one-hot matrices per (class, side) (host-built).

    ohst[side]: [m, te] broadcast of per-group xr rows to te slots (lhsT).
    ohsc[side]: [te, m] segment-sum one-hot for the batched scatter (lhsT).
    Concatenation order matches the edge-pass const-offset bookkeeping:
    class-major, side advances when that (class, side) finishes.
    """
    ohst = [[], []]
    ohsc = [[], []]
    for c in classes:
        for side in (0, 1):
            K = c["KA"] if side == 0 else c["KB"]
            m = c["mA"] if side == 0 else c["mB"]
            te = c["teA"] if side == 0 else c["teB"]
            if te == 0:
                continue
            st = np.zeros((128, te), dtype=np.float32)
            sc = np.zeros((128, m), dtype=np.float32)
            for g in range(m):
                st[g, g * K:(g + 1) * K] = 1.0
                sc[g * K:(g + 1) * K, g] = 1.0
            ohst[side].append(st)
            ohsc[side].append(sc)
    catb = lambda ls: np.concatenate(ls, axis=1).astype(ml_dtypes.bfloat16)
    catf = lambda ls: np.concatenate(ls, axis=1).astype(np.float32)
    return catb(ohst[0]), catb(ohst[1]), catf(ohsc[0]), catf(ohsc[1])


# --------------------------------------------------------------------------
# numpy mock of the exact device pipeline (bf16 rounding included)
# --------------------------------------------------------------------------

def mock_run(meta, W, x):
    bf = ml_dtypes.bfloat16
    f32 = np.float32
    NS, NSP = meta["NS"], meta["NSP"]
    classes = meta["classes"]
    halfrows = (NCORE // 2) * NSP

    # dense-all: full table, computed identically on every device
    xs_all = np.concatenate([meta["slot_x"](d, x) for d in range(NCORE)], axis=0)
    xb = xs_all.astype(bf).astype(f32)
    dall = (xb @ np.asarray(W["wall"], f32))  # psum f32 from bf16 inputs
    table_att = dall[:, :65].astype(bf).astype(f32)  # [8NSP, 65]
    table_yz = dall[:, 65:69].astype(f32)            # [8NSP, 4]
    # patch zero rows p-col
    for r in (0, halfrows):
        table_att[r] = 0.0
        table_att[r, 64] = PCOL_NEG
        table_yz[r] = 0.0

    xr_rows = {}
    for d in range(NCORE):
        xsd = meta["slot_x"](d, x).astype(bf).astype(f32)
        xr_rows[d] = (xsd @ np.asarray(W["wxr"], f32)).astype(bf).astype(f32)

    def unwrap(idx16, n_tot, off):
        w = idx16[:, off // 16:(off + n_tot) // 16]
        return w[:16].T.reshape(-1).astype(np.int64)

    def edge_pass(tab_att, tab_val, xrL, NP_, NF, NVAL):
        # tab_att: [8NSP, NF+1] f32 (bf16-rounded), tab_val: [8NSP, NVAL]
        NV = NVAL + 1  # + denominator (s)
        aggs = [np.zeros((NSP, NV), f32) for _ in range(NCORE)]
        for d in range(NCORE):
            offs = [0, 0]
            for c, side, t0, nch in meta["blocks"]:
                K = c["KA"] if side == 0 else c["KB"]
                m = c["mA"] if side == 0 else c["mB"]
                te = c["teA"] if side == 0 else c["teB"]
                slot0 = c["slot0"]
                idx = meta["idxA"] if side == 0 else meta["idxB"]
                base = 0 if side == 0 else halfrows
                n_tot = 128 * nch
                ii = unwrap(idx[d], n_tot, offs[side]) + base
                offs[side] += n_tot
                att = tab_att[ii].reshape(nch, 128, -1)
                val = tab_val[ii].reshape(nch, 128, -1)
                g_of = np.arange(te) // K
                xr_t = np.stack(
                    [xrL[d][slot0 + (t0 + ch) * m: slot0 + (t0 + ch + 1) * m]
                     for ch in range(nch)], axis=0)  # [nch, m, NF+1]
                H = att[:, :te, :].astype(f32) + xr_t[:, g_of]
                relu = np.maximum(H[:, :, :NF], 0.0).astype(bf).astype(f32)
                dpos = relu[:, :, :NP_].sum(2) - relu[:, :, NP_:].sum(2)
                sv = np.exp(0.8 * (dpos + H[:, :, NF]))
                sval = np.concatenate(
                    [sv[:, :, None] * val[:, :te], sv[:, :, None]], axis=2)
                for ch in range(nch):
                    sl = slot0 + (t0 + ch) * m
                    for g in range(m):
                        aggs[d][sl + g] += sval[ch, g * K:(g + 1) * K].sum(0)
        return aggs

    xr1 = {d: np.zeros((NSP, 65), f32) for d in range(NCORE)}
    for d in range(NCORE):
        xr1[d][:] = xr_rows[d]
    aggs = edge_pass(table_att, table_yz, xr1, W["npos1"], 64, 4)

    # final-pass post-processing -> l2 tables
    l2_att = np.zeros((NCORE * NSP, 3), f32)
    l2_val = np.zeros((NCORE * NSP, 2), f32)
    xr2 = {}
    for d in range(NCORE):
        rcp = 1.0 / (aggs[d][:, 4] + 1e-30)
        sc = aggs[d][:, :4] * rcp[:, None]
        xl2 = sc[:, 0:2] + W["cy"][None, :]
        xr2v = sc[:, 2:4] + W["cz"][None, :]
        p2 = 0.25 * (xl2 @ W["att2"])
        q2 = 0.25 * (xr2v @ W["att2"])
        la = np.zeros((NSP, 3), f32)
        la[:, 0:2] = (xl2[:, W["perm2"]] * W["a2p"][None, :]).astype(bf)
        la[:, 2] = p2.astype(bf)
        lv = xl2.astype(f32)
        la[0] = 0.0
        la[0, 2] = PCOL_NEG
        lv[0] = 0.0
        l2_att[d * NSP:(d + 1) * NSP] = la.astype(bf)
        l2_val[d * NSP:(d + 1) * NSP] = lv
        x2 = np.zeros((NSP, 3), f32)
        x2[:, 0:2] = (xr2v[:, W["perm2"]] * W["a2p"][None, :]).astype(bf)
        x2[:, 2] = q2.astype(bf)
        xr2[d] = x2

    aggs2 = edge_pass(l2_att.astype(bf).astype(f32), l2_val, xr2,
                      W["npos2"], 2, 2)
    out = np.zeros((N, FOUT), dtype=f32)
    for d in range(NCORE):
        rcp = 1.0 / (aggs2[d][:, 2] + 1e-30)
        o2 = aggs2[d][:, :2] * rcp[:, None] + W["b2"][None, :]
        valid = meta["node_at"][d] >= 0
        out[meta["node_at"][d][valid]] = o2[valid]
    return out


# --------------------------------------------------------------------------
# device program (Bass/Tile)
# --------------------------------------------------------------------------

import concourse.bass as bass
import concourse.bacc as bacc_mod
import concourse.mybir as mybir
import concourse.tile as tile
from concourse.bass_utils import run_bass_kernel_spmd


F32 = mybir.dt.float32
BF16 = mybir.dt.bfloat16
U16 = mybir.dt.uint16
I16 = mybir.dt.int16
AF = mybir.ActivationFunctionType
ALU = mybir.AluOpType
AX = mybir.AxisListType


def dense_colmap(nrows):
    """xvT column -> table row map making dense DMA writes contiguous:
    within each DCH-chunk batch, column c*128+p holds table row p*ncch+c."""
    cm = np.empty(nrows, dtype=np.int64)
    nch = nrows // 128
    for cb in range(_ceil(nch, DCH)):
        ncch = min(DCH, nch - cb * DCH)
        base = cb * DCH * 128
        c = np.arange(ncch)[:, None]
        p = np.arange(128)[None, :]
        cm[base + (c * 128 + p).ravel()] = base + (p * ncch + c).ravel()
    return cm


def build_program(meta, W):
    classes = meta["classes"]
    blocks = meta["blocks"]
    NS = meta["NS"]
    NSP = meta["NSP"]
    halfrows = (NCORE // 2) * NSP
    ALLROWS = NCORE * NSP
    NCH = ALLROWS // 128          # dense chunks
    npos1, nneg1 = W["npos1"], 64 - W["npos1"]
    npos2, nneg2 = W["npos2"], 2 - W["npos2"]
    perm2, a2p = W["perm2"], W["a2p"]
    cy, cz = W["cy"], W["cz"]
    att2, b2 = W["att2"], W["b2"]

    ohstA_np, ohstB_np, ohscA_np, ohscB_np = make_oh_consts(classes)
    IDXWA = meta["idxA"].shape[2]
    IDXWB = meta["idxB"].shape[2]

    nc = bacc_mod.Bacc(None)
    xvT = nc.declare_dram_parameter("xvT", [FIN, ALLROWS], BF16, isOutput=False)
    xrT = nc.declare_dram_parameter("xrT", [FIN, NSP], BF16, isOutput=False)
    wallp = nc.declare_dram_parameter("wall", [FIN, 69], BF16, isOutput=False)
    wxrp = nc.declare_dram_parameter("wxr", [FIN, 65], BF16, isOutput=False)
    idxAp = nc.declare_dram_parameter("idxA", [128, IDXWA], I16, isOutput=False)
    idxBp = nc.declare_dram_parameter("idxB", [128, IDXWB], I16, isOutput=False)
    eyep = nc.declare_dram_parameter("eye", [128, 128], BF16, isOutput=False)
    ohstAp = nc.declare_dram_parameter("ohstA", [128, ohstA_np.shape[1]], BF16, isOutput=False)
    ohstBp = nc.declare_dram_parameter("ohstB", [128, ohstB_np.shape[1]], BF16, isOutput=False)
    ohscAp = nc.declare_dram_parameter("ohscA", [128, ohscA_np.shape[1]], F32, isOutput=False)
    ohscBp = nc.declare_dram_parameter("ohscB", [128, ohscB_np.shape[1]], F32, isOutput=False)
    out2 = nc.declare_dram_parameter("out2", [NSP, 2], F32, isOutput=True)

    with tile.TileContext(nc) as tc:
        with (
            tc.tile_pool(name="dram", bufs=1, space="DRAM") as dram,
            tc.tile_pool(name="cpool", bufs=1) as cpool,
            tc.tile_pool(name="dn", bufs=int(os.environ.get("GAT_DNB", 3))) as dn,
            tc.tile_pool(name="sb", bufs=int(os.environ.get("GAT_SBB", 3))) as sb,
            tc.tile_pool(name="sb2", bufs=int(os.environ.get("GAT_SB2", 3))) as sb2,
            tc.tile_pool(name="ps", bufs=int(os.environ.get("GAT_PSB", 2)), space="PSUM") as ps,
            tc.tile_pool(name="psS", bufs=int(os.environ.get("GAT_PSS", 2)), space="PSUM") as psSp,
        ):
            table = dram.tile([ALLROWS, ROW1_U16], U16)
            l2slice = dram.tile([NSP, ROW2_USED], U16)
            l2gath = dram.tile([ALLROWS, ROW2_USED], U16, addr_space="Shared")
            l2tab = dram.tile([ALLROWS, ROW2_U16], U16)
            xr_dram = dram.tile([NSP, XR1_U16], U16)
            xr2_dram = dram.tile([NSP, XR2_U16], U16)
            # per-side aggregate accumulators, node-slot order
            X1 = [dram.tile([NSP, 5], F32, name=f"Xa{s}") for s in range(2)]
            X2 = [dram.tile([NSP, 3], F32, name=f"Xb{s}") for s in range(2)]

            # ---------------- consts ----------------
            wall_sb = cpool.tile([FIN, 69], BF16)
            nc.sync.dma_start(wall_sb[:, :], wallp[:, :])
            wxr_sb = cpool.tile([FIN, 65], BF16)
            nc.sync.dma_start(wxr_sb[:, :], wxrp[:, :])
            eye_sb = cpool.tile([128, 128], BF16)
            nc.sync.dma_start(eye_sb[:, :], eyep[:, :])
            ohstA_sb = cpool.tile([128, ohstA_np.shape[1]], BF16)
            nc.sync.dma_start(ohstA_sb[:, :], ohstAp[:, :])
            ohstB_sb = cpool.tile([128, ohstB_np.shape[1]], BF16)
            nc.sync.dma_start(ohstB_sb[:, :], ohstBp[:, :])
            ohscA_sb = cpool.tile([128, ohscA_np.shape[1]], F32)
            nc.sync.dma_start(ohscA_sb[:, :], ohscAp[:, :])
            ohscB_sb = cpool.tile([128, ohscB_np.shape[1]], F32)
            nc.sync.dma_start(ohscB_sb[:, :], ohscBp[:, :])
            # whole-pass index arrays stay resident in SBUF
            idxA_sb = cpool.tile([128, IDXWA], I16)
            nc.sync.dma_start(idxA_sb[:, :], idxAp[:, :])
            idxB_sb = cpool.tile([128, IDXWB], I16)
            nc.sync.dma_start(idxB_sb[:, :], idxBp[:, :])

            # zero-init the per-side accumulators (K=0 class-sides never
            # flush their ranges; the final pass reads everything)
            NXC = NSP // 128
            zx = cpool.tile([128, NXC * 5], F32)
            nc.vector.memset(zx[:, :], 0)
            for Xs, nv in ((X1, 5), (X2, 3)):
                for s in range(2):
                    nc.sync.dma_start(
                        Xs[s][:, :].rearrange("(c p) v -> p c v", p=128),
                        zx[:, 0:NXC * nv].rearrange("p (c v) -> p c v", v=nv))

            # ---------------- dense-all: full L1 table ----------------
            for cb in range(_ceil(NCH, DCH)):
                ncch = min(DCH, NCH - cb * DCH)
                xin = dn.tile([FIN, DCH * 128], BF16, tag="xin")
                nc.sync.dma_start(xin[:, 0:ncch * 128],
                                  xvT[:, cb * DCH * 128:(cb * DCH + ncch) * 128])
                rows = dn.tile([128, DCH * ROW1_U16], U16, tag="rows")
                rv16 = rows.bitcast(BF16).rearrange("p (c w) -> p c w", w=ROW1_U16)
                rv32 = rows.bitcast(F32).rearrange("p (c w) -> p c w", w=ROW1_U16 // 2)
                for c in range(ncch):
                    ps_d = ps.tile([128, 69], F32, tag="psH0")
                    nc.tensor.matmul(out=ps_d[:, 0:69], lhsT=xin[:, c * 128:(c + 1) * 128],
                                     rhs=wall_sb[:, :], start=True, stop=True)
                    if c % 2 == 0:
                        nc.scalar.activation(rv16[:, c, 0:65], ps_d[:, 0:65], AF.Copy)
                    else:
                        nc.vector.tensor_copy(rv16[:, c, 0:65], ps_d[:, 0:65])
                    nc.vector.tensor_copy(rv32[:, c, ROW1_F32O:ROW1_F32O + 4],
                                          ps_d[:, 65:69])
                nc.gpsimd.dma_start(
                    table[cb * DCH * 128:(cb * DCH + ncch) * 128, :]
                    .rearrange("(p c) w -> p c w", c=ncch),
                    rows.rearrange("p (c w) -> p c w", w=ROW1_U16)[:, 0:ncch, :])

            # patch the two zero rows (slot 0 of dev 0 and dev 4)
            zr = sb.tile([1, ROW1_U16], U16, tag="zr")
            nc.vector.memset(zr[:, :], 0)
            nc.vector.memset(zr.bitcast(BF16)[:, 64:65], PCOL_NEG)
            nc.sync.dma_start(table[0:1, :], zr[:, :])
            nc.sync.dma_start(table[halfrows:halfrows + 1, :], zr[:, :])

            # ---------------- xr rows (own slots only) ----------------
            NXCH = NSP // 128
            for cb in range(_ceil(NXCH, DCH)):
                nxch = min(DCH, NXCH - cb * DCH)
                xin = dn.tile([FIN, DCH * 128], BF16, tag="xrin")
                nc.sync.dma_start(xin[:, 0:nxch * 128],
                                  xrT[:, cb * DCH * 128:(cb * DCH + nxch) * 128])
                xrr = dn.tile([128, DCH * XR1_U16], U16, tag="xrr")
                xv16 = xrr.bitcast(BF16).rearrange("p (c w) -> p c w", w=XR1_U16)
                for c in range(nxch):
                    ps_x = ps.tile([128, 65], F32, tag="psH1")
                    nc.tensor.matmul(out=ps_x[:, 0:65], lhsT=xin[:, c * 128:(c + 1) * 128],
                                     rhs=wxr_sb[:, :], start=True, stop=True)
                    if c % 2 == 0:
                        nc.scalar.activation(xv16[:, c, 0:65], ps_x[:, 0:65], AF.Copy)
                    else:
                        nc.vector.tensor_copy(xv16[:, c, 0:65], ps_x[:, 0:65])
                nc.gpsimd.dma_start(
                    xr_dram[cb * DCH * 128:(cb * DCH + nxch) * 128, :]
                    .rearrange("(p c) w -> p c w", c=nxch),
                    xrr.rearrange("p (c w) -> p c w", w=XR1_U16)[:, 0:nxch, :])

            # ---------------- edge pass ----------------
            def edge_pass(lay):
                if lay == 1:
                    ROWW, F32O, XRW = ROW1_U16, ROW1_F32O, XR1_U16
                    tabT, xrTd, Xs = table, xr_dram, X1
                    NP_, NN_, NF, NVAL = npos1, nneg1, 64, 4
                else:
                    ROWW, F32O, XRW = ROW2_U16, ROW2_F32O, XR2_U16
                    tabT, xrTd, Xs = l2tab, xr2_dram, X2
                    NP_, NN_, NF, NVAL = npos2, nneg2, 2, 2
                NV = NVAL + 1
                NLOG = NF + 1
                BAT_L = BAT if lay == 1 else BAT2
                FT_L = PSW - BAT_L + 1
                offs = [0, 0]       # idx offsets (u16 cols/8) per side
                ohst_off = [0, 0]
                ohsc_off = [0, 0]
                st = {}             # per-side window state
                for c, side, t0, nch in blocks:
                    K = c["KA"] if side == 0 else c["KB"]
                    m = c["mA"] if side == 0 else c["mB"]
                    te = c["teA"] if side == 0 else c["teB"]
                    tiles = c["tilesA"] if side == 0 else c["tilesB"]
                    slot0 = c["slot0"]
                    idx_sb = idxA_sb if side == 0 else idxB_sb
                    ohst_sb = ohstA_sb if side == 0 else ohstB_sb
                    ohsc_sb = ohscA_sb if side == 0 else ohscB_sb
                    if t0 == 0:
                        st[side] = dict(fl0=0, psS=None)

                    # gather for this block
                    STt = sb2.tile([128, SBW * ROWW], U16, tag=f"ST{side}")
                    c16 = offs[side] // 16
                    base = 0 if side == 0 else halfrows
                    nc.gpsimd.dma_gather(
                        out_ap=STt[:, 0:nch * ROWW]
                        .rearrange("p (k w) -> p k w", w=ROWW),
                        in_ap=tabT[base:base + halfrows, :],
                        idxs_ap=idx_sb[:, c16:c16 + 8 * nch],
                        num_idxs=128 * nch, num_idxs_reg=128 * nch,
                        elem_size=ROWW, single_packet=False)
                    offs[side] += 128 * nch
                    Sb = STt.bitcast(BF16).rearrange("p (k w) -> p k w", w=ROWW)
                    Sf = STt.bitcast(F32).rearrange("p (k w) -> p k w", w=ROWW // 2)

                    # xr rows for these tiles
                    xrst = sb2.tile([128, SBW * XR1_U16], U16, tag="xrst")
                    xru = xrst.rearrange("p (k w) -> p k w", w=XR1_U16)
                    r0 = slot0 + t0 * m
                    nc.sync.dma_start(
                        xru[0:m, 0:nch, 0:XRW],
                        xrTd[r0:r0 + nch * m, 0:XRW]
                        .rearrange("(c g) w -> g c w", g=m))
                    xrb = xrst.bitcast(BF16).rearrange("p (k w) -> p k w", w=XR1_U16)

                    for b in range(_ceil(nch, BAT_L)):
                        nb = min(BAT_L, nch - b * BAT_L)
                        bs = slice(b * BAT_L, b * BAT_L + nb)
                        psH = ps.tile([128, BAT_L * NLOG], F32,
                                      name=f"psH{side}", tag=f"psH{side}")
                        pHv = psH.rearrange("p (b w) -> p b w", w=NLOG)
                        nc.tensor.matmul(
                            out=pHv[0:te, 0:nb, :],
                            lhsT=eye_sb[0:te, 0:te],
                            rhs=Sb[0:te, bs, 0:NLOG],
                            start=True, stop=False)
                        nc.tensor.matmul(
                            out=pHv[0:te, 0:nb, :],
                            lhsT=ohst_sb[0:m, ohst_off[side]:ohst_off[side] + te],
                            rhs=xrb[0:m, bs, 0:NLOG],
                            start=False, stop=True)
                        Hr = sb.tile([128, BAT_L * NF], BF16,
                                     name=f"Hr{side}", tag=f"Hr{side}")
                        Hv = Hr.rearrange("p (b w) -> p b w", w=NF)
                        nc.scalar.activation(
                            Hv[0:te, 0:nb, :], pHv[0:te, 0:nb, 0:NF], AF.Relu)
                        dt = sb.tile([128, BAT_L], F32,
                                     name=f"dt{side}", tag=f"dt{side}")
                        if NP_ > 0 and NN_ > 0:
                            Ap = sb.tile([128, BAT_L], F32,
                                         name=f"Ap{side}", tag=f"Ap{side}")
                            An = sb.tile([128, BAT_L], F32,
                                         name=f"An{side}", tag=f"An{side}")
                            nc.vector.tensor_reduce(
                                out=Ap[0:te, 0:nb], in_=Hv[0:te, 0:nb, 0:NP_],
                                axis=AX.X, op=ALU.add)
                            nc.vector.tensor_reduce(
                                out=An[0:te, 0:nb], in_=Hv[0:te, 0:nb, NP_:NF],
                                axis=AX.X, op=ALU.add)
                            nc.vector.tensor_tensor(
                                out=dt[0:te, 0:nb], in0=Ap[0:te, 0:nb],
                                in1=An[0:te, 0:nb], op=ALU.subtract)
                        else:
                            nc.vector.tensor_reduce(
                                out=dt[0:te, 0:nb], in_=Hv[0:te, 0:nb, 0:NF],
                                axis=AX.X, op=ALU.add)
                            if NN_ > 0:
                                nc.vector.tensor_scalar(
                                    out=dt[0:te, 0:nb], in0=dt[0:te, 0:nb],
                                    scalar1=-1.0, scalar2=None, op0=ALU.mult)
                        ep = sb.tile([128, BAT_L], F32,
                                     name=f"ep{side}", tag=f"ep{side}")
                        nc.vector.tensor_tensor(
                            out=ep[0:te, 0:nb], in0=dt[0:te, 0:nb],
                            in1=pHv[0:te, 0:nb, NF], op=ALU.add)
                        # sval = [s * vals, s]; exp lands directly in the s col
                        sval = sb.tile([128, BAT_L * NV], F32,
                                       name=f"sv{side}", tag=f"sv{side}")
                        svt = sval.rearrange("p (b v) -> p b v", v=NV)
                        nc.scalar.activation(
                            svt[0:te, 0:nb, NV - 1], ep[0:te, 0:nb], AF.Exp,
                            scale=0.8)
                        nc.vector.tensor_tensor(
                            out=svt[0:te, 0:nb, 0:NVAL],
                            in0=Sf[0:te, bs, F32O:F32O + NVAL],
                            in1=svt[0:te, 0:nb, NV - 1:NV]
                            .to_broadcast([te, nb, NVAL]),
                            op=ALU.mult)

                        # batched scatter into this side's psum window
                        jj = t0 + b * BAT_L - st[side]["fl0"]
                        if jj == 0:
                            st[side]["psS"] = psSp.tile(
                                [128, PSW * NV], F32, name=f"psS{side}",
                                tag=f"psS{side}")
                        psS = st[side]["psS"]
                        nc.tensor.matmul(
                            out=psS[0:m, jj * NV:(jj + nb) * NV],
                            lhsT=ohsc_sb[0:te, ohsc_off[side]:ohsc_off[side] + m],
                            rhs=sval[0:te, 0:nb * NV],
                            start=True, stop=True)

                        tg_last = t0 + b * BAT_L + nb - 1
                        if jj + nb >= FT_L or tg_last == tiles - 1:
                            flush(Xs[side], NV, m, slot0, st[side]["fl0"],
                                  jj + nb, psS)
                            st[side]["fl0"] = tg_last + 1
                            st[side]["psS"] = None

                    if t0 + nch >= tiles:  # class-side done: advance consts
                        ohst_off[side] += te
                        ohsc_off[side] += m

            def flush(Xsd, NV, m, slot0, f_t0, ntl, psS):
                # PSUM -> SBUF -> DRAM accumulator (node-slot order)
                sc = sb.tile([128, PSW * NV], F32, name="sc", tag="sc")
                nc.vector.tensor_copy(sc[0:m, 0:ntl * NV], psS[0:m, 0:ntl * NV])
                r0 = slot0 + f_t0 * m
                nc.scalar.dma_start(
                    Xsd[r0:r0 + ntl * m, :].rearrange("(j p) v -> p j v", p=m),
                    sc.rearrange("p (j v) -> p j v", v=NV)[0:m, 0:ntl, :])

            # ------------- final pass: combine sides, normalize -------------
            def final_pass(lay):
                Xs = X1 if lay == 1 else X2
                NV = 5 if lay == 1 else 3
                for cb in range(_ceil(NXCH, FCH)):
                    nfc = min(FCH, NXCH - cb * FCH)
                    r0 = cb * FCH * 128
                    ta = sb.tile([128, FCH * NV], F32, tag="fta")
                    tb = sb.tile([128, FCH * NV], F32, tag="ftb")
                    tav = ta.rearrange("p (c v) -> p c v", v=NV)
                    tbv = tb.rearrange("p (c v) -> p c v", v=NV)
                    for s, tv in ((0, tav), (1, tbv)):
                        nc.sync.dma_start(
                            tv[:, 0:nfc, :],
                            Xs[s][r0:r0 + nfc * 128, :]
                            .rearrange("(c p) v -> p c v", p=128))
                    tt = sb.tile([128, FCH * NV], F32, tag="ftt")
                    ttv = tt.rearrange("p (c v) -> p c v", v=NV)
                    nc.vector.tensor_tensor(
                        out=ttv[:, 0:nfc, :], in0=tav[:, 0:nfc, :],
                        in1=tbv[:, 0:nfc, :], op=ALU.add)
                    rcpi = sb.tile([128, FCH], F32, tag="rcpi")
                    nc.vector.tensor_scalar(
                        out=rcpi[:, 0:nfc], in0=ttv[:, 0:nfc, NV - 1],
                        scalar1=1e-30, scalar2=None, op0=ALU.add)
                    rcp = sb.tile([128, FCH], F32, tag="rcp")
                    nc.vector.reciprocal(rcp[:, 0:nfc], rcpi[:, 0:nfc])
                    scv = sb.tile([128, FCH * 4], F32, tag="fsc")
                    sv = scv.rearrange("p (c v) -> p c v", v=4)
                    nc.vector.tensor_tensor(
                        out=sv[:, 0:nfc, 0:NV - 1], in0=ttv[:, 0:nfc, 0:NV - 1],
                        in1=rcp[:, 0:nfc].rearrange("p (c o) -> p c o", o=1)
                        .to_broadcast([128, nfc, NV - 1]),
                        op=ALU.mult)
                    if lay == 1:
                        l2r = sb.tile([128, FCH * ROW2_USED], U16, tag="l2r")
                        lb = l2r.bitcast(BF16).rearrange("p (j v) -> p j v", v=ROW2_USED)
                        lf = l2r.bitcast(F32).rearrange("p (j v) -> p j v", v=ROW2_USED // 2)
                        x2r = sb.tile([128, FCH * XR2_U16], U16, tag="x2r")
                        xb2 = x2r.bitcast(BF16).rearrange("p (j v) -> p j v", v=XR2_U16)
                        t1 = sb.tile([128, FCH], F32, tag="ft1")
                        t2 = sb.tile([128, FCH], F32, tag="ft2")
                        for cc in range(2):
                            nc.vector.tensor_scalar(
                                out=lb[:, 0:nfc, cc], in0=sv[:, 0:nfc, perm2[cc]],
                                scalar1=float(a2p[cc]),
                                scalar2=float(a2p[cc] * cy[perm2[cc]]),
                                op0=ALU.mult, op1=ALU.add)
                            nc.vector.tensor_scalar(
                                out=xb2[:, 0:nfc, cc], in0=sv[:, 0:nfc, 2 + perm2[cc]],
                                scalar1=float(a2p[cc]),
                                scalar2=float(a2p[cc] * cz[perm2[cc]]),
                                op0=ALU.mult, op1=ALU.add)
                        nc.vector.tensor_scalar(
                            out=t1[:, 0:nfc], in0=sv[:, 0:nfc, 0],
                            scalar1=float(0.25 * att2[0]),
                            scalar2=float(0.25 * (att2 @ cy)),
                            op0=ALU.mult, op1=ALU.add)
                        nc.vector.tensor_scalar(
                            out=t2[:, 0:nfc], in0=sv[:, 0:nfc, 1],
                            scalar1=float(0.25 * att2[1]), scalar2=None, op0=ALU.mult)
                        nc.vector.tensor_tensor(
                            out=lb[:, 0:nfc, 2], in0=t1[:, 0:nfc],
                            in1=t2[:, 0:nfc], op=ALU.add)
                        nc.vector.tensor_scalar(
                            out=t1[:, 0:nfc], in0=sv[:, 0:nfc, 2],
                            scalar1=float(0.25 * att2[0]),
                            scalar2=float(0.25 * (att2 @ cz)),
                            op0=ALU.mult, op1=ALU.add)
                        nc.vector.tensor_scalar(
                            out=t2[:, 0:nfc], in0=sv[:, 0:nfc, 3],
                            scalar1=float(0.25 * att2[1]), scalar2=None, op0=ALU.mult)
                        nc.vector.tensor_tensor(
                            out=xb2[:, 0:nfc, 2], in0=t1[:, 0:nfc],
                            in1=t2[:, 0:nfc], op=ALU.add)
                        nc.vector.memset(xb2[:, 0:nfc, 3], 0.0)
                        for cc in range(2):
                            nc.vector.tensor_scalar(
                                out=lf[:, 0:nfc, ROW2_F32O + cc],
                                in0=sv[:, 0:nfc, cc],
                                scalar1=float(cy[cc]), scalar2=None, op0=ALU.add)
                        nc.scalar.dma_start(
                            l2slice[r0:r0 + nfc * 128, :]
                            .rearrange("(j p) w -> p j w", p=128),
                            l2r.rearrange("p (j v) -> p j v", v=ROW2_USED)[
                                :, 0:nfc, :])
                        nc.scalar.dma_start(
                            xr2_dram[r0:r0 + nfc * 128, 0:XR2_U16]
                            .rearrange("(j p) w -> p j w", p=128),
                            x2r.rearrange("p (j v) -> p j v", v=XR2_U16)[:, 0:nfc, :])
                    else:
                        o2 = sb.tile([128, FCH * 2], F32, tag="o2")
                        o2v = o2.rearrange("p (j v) -> p j v", v=2)
                        for cc in range(2):
                            nc.vector.tensor_scalar(
                                out=o2v[:, 0:nfc, cc], in0=sv[:, 0:nfc, cc],
                                scalar1=float(b2[cc]), scalar2=None, op0=ALU.add)
                        nc.scalar.dma_start(
                            out2[r0:r0 + nfc * 128, :].rearrange("(j p) w -> p j w", p=128),
                            o2v[:, 0:nfc, :])

            edge_pass(1)
            final_pass(1)

            # zero row 0 of l2slice (att cols 0, p2 col -100, vals 0)
            zr2 = sb.tile([1, ROW2_USED], U16, tag="zr2")
            nc.vector.memset(zr2[:, :], 0)
            nc.vector.memset(zr2.bitcast(BF16)[:, 2:3], PCOL_NEG)
            nc.sync.dma_start(l2slice[0:1, :], zr2[:, :])

            # AllGather the compact 16B rows, then expand locally into the
            # 256B-stride gatherable table (only the used bytes ever move)
            nc.gpsimd.collective_compute(
                "AllGather", ALU.bypass,
                replica_groups=[list(range(NCORE))],
                ins=[l2slice[:, :]], outs=[l2gath[:, :]],
            )
            nc.sync.dma_start(l2tab[:, 0:ROW2_USED], l2gath[:, :])

            edge_pass(2)
            final_pass(2)

    return nc, NSP


def run_device(meta, W, x, trace=False):
    nc, NSP = build_program(meta, W)
    NS = meta["NS"]
    classes = meta["classes"]
    bf = ml_dtypes.bfloat16
    ohstA, ohstB, ohscA, ohscB = make_oh_consts(classes)

    xs_all = np.concatenate(
        [meta["slot_x"](d, x) for d in range(NCORE)], axis=0).astype(bf)
    xvT_np = np.ascontiguousarray(xs_all[dense_colmap(len(xs_all))].T)

    in_maps = []
    for d in range(NCORE):
        xsd = meta["slot_x"](d, x).astype(bf)
        im = dict(
            xvT=xvT_np,
            xrT=np.ascontiguousarray(xsd[dense_colmap(len(xsd))].T),
            wall=np.asarray(W["wall"]),
            wxr=np.asarray(W["wxr"]),
            idxA=meta["idxA"][d],
            idxB=meta["idxB"][d],
            eye=np.eye(128, dtype=bf),
            ohstA=ohstA, ohstB=ohstB, ohscA=ohscA, ohscB=ohscB,
        )
        in_maps.append(im)

    if not nc.is_finalized():
        nc.finalize()
    res = run_bass_kernel_spmd(nc, in_maps, list(range(NCORE)), trace=trace)
    outs = res.results
    out = np.zeros((N, FOUT), dtype=np.float32)
    for d in range(NCORE):
        o = outs[d]["out2"]
        valid = meta["node_at"][d] >= 0
        out[meta["node_at"][d][valid]] = o[:NSP][valid]
    return out, res


# --------------------------------------------------------------------------
# entry
# --------------------------------------------------------------------------

def kernel(**inputs):
    x = np.asarray(inputs["x"], dtype=np.float32)
    meta = build_schedule(np.asarray(inputs["edge_index"]))
    W = prep_weights(
        np.asarray(inputs["Wl1"], np.float32), np.asarray(inputs["Wr1"], np.float32),
        np.asarray(inputs["att1"], np.float32), np.asarray(inputs["b1"], np.float32),
        np.asarray(inputs["Wl2"], np.float32), np.asarray(inputs["Wr2"], np.float32),
        np.asarray(inputs["att2"], np.float32), np.asarray(inputs["b2"], np.float32),
    )
    if os.environ.get("GAT_MOCK"):
        return mock_run(meta, W, x)
    out, _res = run_device(meta, W, x)
    return out


if __name__ == "__main__":
    pass


# revision 19
# speedup vs baseline: 1.1830x; 1.1830x over previous
"""GATv2 (2-layer, heads=1) on 8 trn2 NeuronCores.

Strategy v2 (dst-sharded, per-side class-scheduled):
  - Nodes partitioned across 8 cores by dst id (6250 each). Edges (with self
    loops) grouped by (dst, side) where side A = src on devices 0-3, side B =
    src on devices 4-7 (keeps gather indices < 32768 for int16 dma_gather).
  - Pair-classes (KA, KB) = 4-padded side-degrees fix the node ordering, but
    each side is tiled INDEPENDENTLY: side s of a class uses m_s = 128//K_s
    dst-groups per tile (te_s = m_s*K_s slots), so both sides pack ~94% of
    the 128 gather slots (v1 paired tiles wasted the smaller side).
  - Every gather descriptor fetches a real row; padding slots fetch row 0
    whose p-column is -100 so exp() kills them.
  - No layer-1 AllGather: every core computes the FULL layer-1 node table
    locally from the replicated (slot-ordered) x, in bf16.
  - Layer-1 logit:  e = att.leaky_relu(xl[src]+xr[dst])
                    = 0.8*((p_j+q_i)/4 + relu-sum over sign-sorted, |att|-
                      scaled feature columns), with p=att.xl/4, q=att.xr/4.
    Softmax max-subtraction is skipped (shift-invariant; exponents small).
  - h (layer-1 output) is never materialized: layer 2 only needs
    xl2 = h@Wl2, xr2 = h@Wr2, linear in the layer-1 aggregates of
    y = xl@Wl2, z = xl@Wr2 (2 cols each).
  - Aggregation: s = exp(e) per slot; values scaled by s on DVE, then ONE
    batched matmul per (batch, side) with a constant per-class one-hot lhsT
    accumulates [m_s, NV] per tile into that side's PSUM window. Windows
    flush (PSUM -> SBUF copy -> DMA) into per-side DRAM accumulators X_s in
    node-slot order; a vectorized final pass adds the sides, normalizes and
    emits the layer-2 tables / output.
  - Layer-2 table (16B/row used) is shared via ONE strided AllGather that
    moves only the used bytes; rows keep 256B stride for dma_gather.
  - Both passes use the same schedule, table layout and index arrays.
"""

import os
import numpy as np
import ml_dtypes

N = 50000
E = 800000
FIN = 128
HID = 64
FOUT = 2
NCORE = 8
NPD = N // NCORE  # 6250 dst nodes per device
NHALF = N // 2    # src < NHALF -> side A

SBW = 48   # tiles (chunks) per gather super-call
BAT = 7    # tiles per matmul batch (layer 1; 65 f32 cols cap PSUM at 7)
BAT2 = 48  # tiles per matmul batch (layer 2; rows are tiny)
PSW = 97   # psum window capacity in tiles (97*5*4B = 1940B <= 2KB)
DCH = 14   # dense chunks per DMA write batch (2 psum batches)
FCH = 16   # final-pass chunks per iteration

ROW1_U16 = 128  # L1 table row: 65 bf16 | pad | 4 f32 (yz) | pad (256B)
ROW1_F32O = 33  # f32 view offset of [y0 y1 z0 z1]
ROW2_U16 = 128  # L2 table row: 3 bf16 | pad | 2 f32 | pad (256B)
ROW2_F32O = 2   # f32 view offset of [xl2_0 xl2_1]
ROW2_USED = 8   # u16 cols actually used (16B) -> strided AllGather width
XR1_U16 = 66    # xr row: 65 bf16 | pad
XR2_U16 = 4     # xr2 row: 3 bf16 | pad
PCOL_NEG = -100.0  # p-col of zero rows: exp(0.8*(x - 100)) ~ 0


def _ceil(a, b):
    return -(-a // b)


def _class_of(deg):
    # pad side-degree to a multiple of 4 (0 stays 0)
    return np.where(deg > 0, np.maximum(np.ceil(deg / 4).astype(np.int64) * 4, 4), 0)


def build_schedule(edge_index):
    """Host-side scheduling. Pure numpy. Returns meta dict.

    Device assignment: side A/B of an edge is fixed by src id (< NHALF).
    Within each id-half, nodes are assigned to that half's 4 devices
    round-robin per pair-class, so per-device class counts are balanced
    (tiles = max over devices is tight)."""
    src = np.asarray(edge_index[0], dtype=np.int64)
    dst = np.asarray(edge_index[1], dtype=np.int64)
    loops = np.arange(N, dtype=np.int64)
    src = np.concatenate([src, loops])
    dst = np.concatenate([dst, loops])

    sideB = (src >= NHALF).astype(np.int64)  # 0 = A, 1 = B

    # per-(node, side) degree (global, assignment-independent)
    degA = np.bincount(dst[sideB == 0], minlength=N)
    degB = np.bincount(dst[sideB == 1], minlength=N)
    # Reserve a self-loop slot on BOTH sides before class quantization: the
    # self-loop only lands on the node's own half, which would otherwise
    # shift ~27% of nodes across the 4-multiple class boundary differently
    # per half -- and SPMD tiling pays the max over devices.
    inB = (np.arange(N) >= NHALF).astype(np.int64)
    KA_n = _class_of(degA + inB * (degA > 0))  # [N]
    KB_n = _class_of(degB + (1 - inB) * (degB > 0))
    assert max(KA_n.max(), KB_n.max()) <= 124

    pair_n = KA_n * 1000 + KB_n
    pairs = np.unique(pair_n)

    # class-balanced device assignment within each id-half
    dev_of = np.empty(N, dtype=np.int64)
    for h in range(2):
        base = h * NHALF
        for pair in pairs:
            ids = np.where(pair_n[base:base + NHALF] == pair)[0] + base
            dev_of[ids] = h * 4 + np.arange(len(ids)) % 4

    # pair-class list with per-side tiling; tiles from max over devices
    classes = []
    for pair in pairs:
        KA, KB = int(pair // 1000), int(pair % 1000)
        sel = pair_n == pair
        n_k = np.bincount(dev_of[sel], minlength=NCORE)
        nmax = int(n_k.max())
        if nmax == 0:
            continue
        mA = 128 // KA if KA else 0
        mB = 128 // KB if KB else 0
        tA = _ceil(nmax, mA) if KA else 0
        tB = _ceil(nmax, mB) if KB else 0
        span = max(tA * mA if KA else 0, tB * mB if KB else 0)
        classes.append(dict(KA=KA, KB=KB, mA=mA, mB=mB,
                            teA=mA * KA, teB=mB * KB,
                            tilesA=tA, tilesB=tB, span=span))

    # slot layout: slot 0 reserved (zero row); then per class, node-linear
    ns = 1
    for c in classes:
        c["slot0"] = ns
        ns += c["span"]
    NS = ns
    NSP = -(-NS // 128) * 128
    assert 4 * NSP <= 32768, NSP  # int16 gather index limit

    # per-device slot assignment (global node ids)
    slot_glob = np.full(N, -1, dtype=np.int64)   # node -> slot on its device
    node_at = np.full((NCORE, NSP), -1, dtype=np.int64)  # (dev, slot) -> node
    for c in classes:
        pair = c["KA"] * 1000 + c["KB"]
        ids = np.where(pair_n == pair)[0]
        for d in range(NCORE):
            idd = ids[dev_of[ids] == d]
            s = c["slot0"] + np.arange(len(idd))
            slot_glob[idd] = s
            node_at[d, s] = idd

    # table row of a node: dev*NSP + slot (same for L1 table and L2 table)
    halfrows = (NCORE // 2) * NSP
    grow = dev_of[src] * NSP + slot_glob[src]

    # blocks: side-major supers of <= SBW tiles; a super's gather spans
    # class boundaries (members), one dma_gather per super
    blocks = []
    for side in (0, 1):
        cur, used = [], 0
        for c in classes:
            tiles = c["tilesA"] if side == 0 else c["tilesB"]
            t0 = 0
            while t0 < tiles:
                nch = min(SBW - used, tiles - t0)
                cur.append((c, t0, nch))
                used += nch
                t0 += nch
                if used == SBW:
                    blocks.append((side, cur))
                    cur, used = [], 0
        if cur:
            blocks.append((side, cur))

    # edge -> (class, side, tile, row-in-tile)
    eslot = dev_of[dst] * NS + slot_glob[dst]
    okey = eslot * 2 + sideB
    order = np.argsort(okey, kind="stable")
    es = eslot[order]
    sb = sideB[order]
    gr = grow[order]
    uniq, start_idx, counts = np.unique(okey[order], return_index=True,
                                        return_counts=True)
    pos = np.arange(len(es)) - np.repeat(start_idx, counts)
    e_dev = es // NS
    e_slot = es % NS

    cls_of_slot = np.zeros(NS, dtype=np.int64)
    for ci, c in enumerate(classes):
        cls_of_slot[c["slot0"]: c["slot0"] + c["span"]] = ci
    cKA = np.array([c["KA"] for c in classes])
    cKB = np.array([c["KB"] for c in classes])
    cmA = np.array([max(c["mA"], 1) for c in classes])
    cmB = np.array([max(c["mB"], 1) for c in classes])
    cs0 = np.array([c["slot0"] for c in classes])
    ci = cls_of_slot[e_slot]
    rel = e_slot - cs0[ci]
    cm = np.where(sb == 0, cmA[ci], cmB[ci])
    cK = np.where(sb == 0, cKA[ci], cKB[ci])
    tt = rel // cm
    gg = rel % cm
    rr = gg * cK + pos
    assert (pos < cK).all()

    # per-super index arrays (wrapped int16), sides separate
    idx_parts = [[], []]  # [A parts, B parts]
    for side, members in blocks:
        for c, t0, nch in members:
            cidx = classes.index(c)
            n_tot = 128 * nch
            val = np.zeros((NCORE, n_tot), dtype=np.int64)
            msk = (ci == cidx) & (sb == side) & (tt >= t0) & (tt < t0 + nch)
            n_e = (tt[msk] - t0) * 128 + rr[msk]
            rowv = gr[msk]
            if side == 1:
                rowv = rowv - halfrows
            if len(rowv):
                assert rowv.min() >= 0 and rowv.max() < halfrows
            val[e_dev[msk], n_e] = rowv
            # wrap int16: n -> [n%16, n//16], replicated over 8 core-groups
            w = val.reshape(NCORE, n_tot // 16, 16).transpose(0, 2, 1)
            idx_parts[side].append(np.tile(w, (1, 8, 1)).astype(np.int16))
    idxA = np.concatenate(idx_parts[0], axis=2)
    idxB = np.concatenate(idx_parts[1], axis=2)

    # x rows in slot order, per device (node_at holds global node ids)
    def slot_x(d, x):
        xs = np.zeros((NSP, FIN), dtype=np.float32)
        valid = node_at[d] >= 0
        xs[valid] = x[node_at[d][valid]]
        return xs

    meta = dict(classes=classes, blocks=blocks, NS=NS, NSP=NSP,
                node_at=node_at, idxA=idxA, idxB=idxB, slot_x=slot_x)
    return meta


def prep_weights(Wl1, Wr1, att1, b1, Wl2, Wr2, att2, b2):
    """Host weight transforms. Returns dict of constant arrays."""
    bf = ml_dtypes.bfloat16
    perm1 = np.argsort(att1 < 0, kind="stable")
    a1p = np.abs(att1[perm1])
    npos1 = int((att1 >= 0).sum())
    wtab_att = np.zeros((FIN, 65), dtype=np.float32)
    wtab_att[:, :64] = Wl1[:, perm1] * a1p[None, :]
    wtab_att[:, 64] = 0.25 * (Wl1 @ att1)
    w_yz = np.concatenate([Wl1 @ Wl2, Wl1 @ Wr2], axis=1).astype(np.float32)
    wxr = np.zeros((FIN, 65), dtype=np.float32)
    wxr[:, :64] = Wr1[:, perm1] * a1p[None, :]
    wxr[:, 64] = 0.25 * (Wr1 @ att1)
    # fused dense rhs: [att 65 | yz 4] bf16
    wall = np.concatenate([wtab_att, w_yz], axis=1).astype(bf)

    cy = b1 @ Wl2  # [2]
    cz = b1 @ Wr2
    perm2 = np.argsort(att2 < 0, kind="stable")
    a2p = np.abs(att2[perm2])
    npos2 = int((att2 >= 0).sum())
    return dict(
        wall=wall, wxr=wxr.astype(bf), npos1=npos1,
        cy=cy, cz=cz, perm2=perm2, a2p=a2p, npos2=npos2,
        att2=att2, b2=b2,
    )


def make_oh_consts(classes):
    """Constant one-hot matrices per (class, side) (host-built).

    ohst[side]: [m, te] broadcast of per-group xr rows to te slots (lhsT).
    ohsc[side]: [te, m] segment-sum one-hot for the batched scatter (lhsT).
    Concatenation order matches the edge-pass const-offset bookkeeping:
    class-major, side advances when that (class, side) finishes.
    """
    ohst = [[], []]
    ohsc = [[], []]
    for c in classes:
        for side in (0, 1):
            K = c["KA"] if side == 0 else c["KB"]
            m = c["mA"] if side == 0 else c["mB"]
            te = c["teA"] if side == 0 else c["teB"]
            if te == 0:
                continue
            st = np.zeros((128, te), dtype=np.float32)
            sc = np.zeros((128, m), dtype=np.float32)
            for g in range(m):
                st[g, g * K:(g + 1) * K] = 1.0
                sc[g * K:(g + 1) * K, g] = 1.0
            ohst[side].append(st)
            ohsc[side].append(sc)
    catb = lambda ls: np.concatenate(ls, axis=1).astype(ml_dtypes.bfloat16)
    catf = lambda ls: np.concatenate(ls, axis=1).astype(np.float32)
    return catb(ohst[0]), catb(ohst[1]), catf(ohsc[0]), catf(ohsc[1])


# --------------------------------------------------------------------------
# numpy mock of the exact device pipeline (bf16 rounding included)
# --------------------------------------------------------------------------

def mock_run(meta, W, x):
    bf = ml_dtypes.bfloat16
    f32 = np.float32
    NS, NSP = meta["NS"], meta["NSP"]
    classes = meta["classes"]
    halfrows = (NCORE // 2) * NSP

    # dense-all: full table, computed identically on every device
    xs_all = np.concatenate([meta["slot_x"](d, x) for d in range(NCORE)], axis=0)
    xb = xs_all.astype(bf).astype(f32)
    dall = (xb @ np.asarray(W["wall"], f32))  # psum f32 from bf16 inputs
    table_att = dall[:, :65].astype(bf).astype(f32)  # [8NSP, 65]
    table_yz = dall[:, 65:69].astype(f32)            # [8NSP, 4]
    # patch zero rows p-col
    for r in (0, halfrows):
        table_att[r] = 0.0
        table_att[r, 64] = PCOL_NEG
        table_yz[r] = 0.0

    xr_rows = {}
    for d in range(NCORE):
        xsd = meta["slot_x"](d, x).astype(bf).astype(f32)
        xr_rows[d] = (xsd @ np.asarray(W["wxr"], f32)).astype(bf).astype(f32)

    def unwrap(idx16, n_tot, off):
        w = idx16[:, off // 16:(off + n_tot) // 16]
        return w[:16].T.reshape(-1).astype(np.int64)

    def edge_pass(tab_att, tab_val, xrL, NP_, NF, NVAL):
        # tab_att: [8NSP, NF+1] f32 (bf16-rounded), tab_val: [8NSP, NVAL]
        NV = NVAL + 1  # + denominator (s)
        aggs = [np.zeros((NSP, NV), f32) for _ in range(NCORE)]
        for d in range(NCORE):
            offs = [0, 0]
            for side, members in meta["blocks"]:
              for c, t0, nch in members:
                K = c["KA"] if side == 0 else c["KB"]
                m = c["mA"] if side == 0 else c["mB"]
                te = c["teA"] if side == 0 else c["teB"]
                slot0 = c["slot0"]
                idx = meta["idxA"] if side == 0 else meta["idxB"]
                base = 0 if side == 0 else halfrows
                n_tot = 128 * nch
                ii = unwrap(idx[d], n_tot, offs[side]) + base
                offs[side] += n_tot
                att = tab_att[ii].reshape(nch, 128, -1)
                val = tab_val[ii].reshape(nch, 128, -1)
                g_of = np.arange(te) // K
                xr_t = np.stack(
                    [xrL[d][slot0 + (t0 + ch) * m: slot0 + (t0 + ch + 1) * m]
                     for ch in range(nch)], axis=0)  # [nch, m, NF+1]
                H = att[:, :te, :].astype(f32) + xr_t[:, g_of]
                relu = np.maximum(H[:, :, :NF], 0.0).astype(bf).astype(f32)
                dpos = relu[:, :, :NP_].sum(2) - relu[:, :, NP_:].sum(2)
                sv = np.exp(0.8 * (dpos + H[:, :, NF]))
                sval = np.concatenate(
                    [sv[:, :, None] * val[:, :te], sv[:, :, None]], axis=2)
                for ch in range(nch):
                    sl = slot0 + (t0 + ch) * m
                    for g in range(m):
                        aggs[d][sl + g] += sval[ch, g * K:(g + 1) * K].sum(0)
        return aggs

    xr1 = {d: np.zeros((NSP, 65), f32) for d in range(NCORE)}
    for d in range(NCORE):
        xr1[d][:] = xr_rows[d]
    aggs = edge_pass(table_att, table_yz, xr1, W["npos1"], 64, 4)

    # final-pass post-processing -> l2 tables
    l2_att = np.zeros((NCORE * NSP, 3), f32)
    l2_val = np.zeros((NCORE * NSP, 2), f32)
    xr2 = {}
    for d in range(NCORE):
        rcp = 1.0 / (aggs[d][:, 4] + 1e-30)
        sc = aggs[d][:, :4] * rcp[:, None]
        xl2 = sc[:, 0:2] + W["cy"][None, :]
        xr2v = sc[:, 2:4] + W["cz"][None, :]
        p2 = 0.25 * (xl2 @ W["att2"])
        q2 = 0.25 * (xr2v @ W["att2"])
        la = np.zeros((NSP, 3), f32)
        la[:, 0:2] = (xl2[:, W["perm2"]] * W["a2p"][None, :]).astype(bf)
        la[:, 2] = p2.astype(bf)
        lv = xl2.astype(f32)
        la[0] = 0.0
        la[0, 2] = PCOL_NEG
        lv[0] = 0.0
        l2_att[d * NSP:(d + 1) * NSP] = la.astype(bf)
        l2_val[d * NSP:(d + 1) * NSP] = lv
        x2 = np.zeros((NSP, 3), f32)
        x2[:, 0:2] = (xr2v[:, W["perm2"]] * W["a2p"][None, :]).astype(bf)
        x2[:, 2] = q2.astype(bf)
        xr2[d] = x2

    aggs2 = edge_pass(l2_att.astype(bf).astype(f32), l2_val, xr2,
                      W["npos2"], 2, 2)
    out = np.zeros((N, FOUT), dtype=f32)
    for d in range(NCORE):
        rcp = 1.0 / (aggs2[d][:, 2] + 1e-30)
        o2 = aggs2[d][:, :2] * rcp[:, None] + W["b2"][None, :]
        valid = meta["node_at"][d] >= 0
        out[meta["node_at"][d][valid]] = o2[valid]
    return out


# --------------------------------------------------------------------------
# device program (Bass/Tile)
# --------------------------------------------------------------------------

import concourse.bass as bass
import concourse.bacc as bacc_mod
import concourse.mybir as mybir
import concourse.tile as tile
from concourse.bass_utils import run_bass_kernel_spmd


F32 = mybir.dt.float32
BF16 = mybir.dt.bfloat16
U16 = mybir.dt.uint16
I16 = mybir.dt.int16
AF = mybir.ActivationFunctionType
ALU = mybir.AluOpType
AX = mybir.AxisListType


def dense_colmap(nrows):
    """xvT column -> table row map making dense DMA writes contiguous:
    within each DCH-chunk batch, column c*128+p holds table row p*ncch+c."""
    cm = np.empty(nrows, dtype=np.int64)
    nch = nrows // 128
    for cb in range(_ceil(nch, DCH)):
        ncch = min(DCH, nch - cb * DCH)
        base = cb * DCH * 128
        c = np.arange(ncch)[:, None]
        p = np.arange(128)[None, :]
        cm[base + (c * 128 + p).ravel()] = base + (p * ncch + c).ravel()
    return cm


def build_program(meta, W):
    classes = meta["classes"]
    blocks = meta["blocks"]
    NS = meta["NS"]
    NSP = meta["NSP"]
    halfrows = (NCORE // 2) * NSP
    ALLROWS = NCORE * NSP
    NCH = ALLROWS // 128          # dense chunks
    npos1, nneg1 = W["npos1"], 64 - W["npos1"]
    npos2, nneg2 = W["npos2"], 2 - W["npos2"]
    perm2, a2p = W["perm2"], W["a2p"]
    cy, cz = W["cy"], W["cz"]
    att2, b2 = W["att2"], W["b2"]

    ohstA_np, ohstB_np, ohscA_np, ohscB_np = make_oh_consts(classes)
    IDXWA = meta["idxA"].shape[2]
    IDXWB = meta["idxB"].shape[2]

    nc = bacc_mod.Bacc(None)
    xvT = nc.declare_dram_parameter("xvT", [FIN, ALLROWS], BF16, isOutput=False)
    xrT = nc.declare_dram_parameter("xrT", [FIN, NSP], BF16, isOutput=False)
    wallp = nc.declare_dram_parameter("wall", [FIN, 69], BF16, isOutput=False)
    wxrp = nc.declare_dram_parameter("wxr", [FIN, 65], BF16, isOutput=False)
    idxAp = nc.declare_dram_parameter("idxA", [128, IDXWA], I16, isOutput=False)
    idxBp = nc.declare_dram_parameter("idxB", [128, IDXWB], I16, isOutput=False)
    eyep = nc.declare_dram_parameter("eye", [128, 128], BF16, isOutput=False)
    ohstAp = nc.declare_dram_parameter("ohstA", [128, ohstA_np.shape[1]], BF16, isOutput=False)
    ohstBp = nc.declare_dram_parameter("ohstB", [128, ohstB_np.shape[1]], BF16, isOutput=False)
    ohscAp = nc.declare_dram_parameter("ohscA", [128, ohscA_np.shape[1]], F32, isOutput=False)
    ohscBp = nc.declare_dram_parameter("ohscB", [128, ohscB_np.shape[1]], F32, isOutput=False)
    out2 = nc.declare_dram_parameter("out2", [NSP, 2], F32, isOutput=True)

    with tile.TileContext(nc) as tc:
        with (
            tc.tile_pool(name="dram", bufs=1, space="DRAM") as dram,
            tc.tile_pool(name="cpool", bufs=1) as cpool,
            tc.tile_pool(name="dn", bufs=int(os.environ.get("GAT_DNB", 3))) as dn,
            tc.tile_pool(name="sb", bufs=int(os.environ.get("GAT_SBB", 3))) as sb,
            tc.tile_pool(name="sb2", bufs=int(os.environ.get("GAT_SB2", 3))) as sb2,
            tc.tile_pool(name="ps", bufs=int(os.environ.get("GAT_PSB", 4)), space="PSUM") as ps,
            tc.tile_pool(name="psS", bufs=int(os.environ.get("GAT_PSS", 2)), space="PSUM") as psSp,
        ):
            table = dram.tile([ALLROWS, ROW1_U16], U16)
            l2slice = dram.tile([NSP, ROW2_USED], U16)
            NXC0 = NSP // 128
            _nfin = _ceil(NXC0, FCH)
            # AllGather pieces: (fire-after-final-chunk, row0, row1)
            _pieces = [(_nfin - 1, 0, NSP)]
            l2gathP = [dram.tile([NCORE * (r1 - r0), ROW2_USED], U16,
                                 addr_space="Shared", name=f"l2g{i}")
                       for i, (_, r0, r1) in enumerate(_pieces)]
            l2tab = dram.tile([ALLROWS, ROW2_U16], U16)
            xr_dram = dram.tile([NSP, XR1_U16], U16)
            xr2_dram = dram.tile([NSP, XR2_U16], U16)
            # per-side aggregate accumulators, node-slot order
            X1 = [dram.tile([NSP, 5], F32, name=f"Xa{s}") for s in range(2)]
            X2 = [dram.tile([NSP, 3], F32, name=f"Xb{s}") for s in range(2)]

            # ---------------- consts ----------------
            wall_sb = cpool.tile([FIN, 69], BF16)
            nc.sync.dma_start(wall_sb[:, :], wallp[:, :])
            wxr_sb = cpool.tile([FIN, 65], BF16)
            nc.sync.dma_start(wxr_sb[:, :], wxrp[:, :])
            eye_sb = cpool.tile([128, 128], BF16)
            nc.sync.dma_start(eye_sb[:, :], eyep[:, :])
            ohstA_sb = cpool.tile([128, ohstA_np.shape[1]], BF16)
            nc.sync.dma_start(ohstA_sb[:, :], ohstAp[:, :])
            ohstB_sb = cpool.tile([128, ohstB_np.shape[1]], BF16)
            nc.sync.dma_start(ohstB_sb[:, :], ohstBp[:, :])
            ohscA_sb = cpool.tile([128, ohscA_np.shape[1]], F32)
            nc.sync.dma_start(ohscA_sb[:, :], ohscAp[:, :])
            ohscB_sb = cpool.tile([128, ohscB_np.shape[1]], F32)
            nc.sync.dma_start(ohscB_sb[:, :], ohscBp[:, :])
            # whole-pass index arrays stay resident in SBUF
            idxA_sb = cpool.tile([128, IDXWA], I16)
            nc.sync.dma_start(idxA_sb[:, :], idxAp[:, :])
            idxB_sb = cpool.tile([128, IDXWB], I16)
            nc.sync.dma_start(idxB_sb[:, :], idxBp[:, :])

            # zero-init the per-side accumulators (K=0 class-sides never
            # flush their ranges; the final pass reads everything)
            NXC = NSP // 128
            zx = cpool.tile([128, NXC * 5], F32)
            nc.vector.memset(zx[:, :], 0)
            for Xs, nv in ((X1, 5), (X2, 3)):
                for s in range(2):
                    nc.sync.dma_start(
                        Xs[s][:, :].rearrange("(c p) v -> p c v", p=128),
                        zx[:, 0:NXC * nv].rearrange("p (c v) -> p c v", v=nv))

            # ---------------- dense-all: full L1 table ----------------
            for cb in range(_ceil(NCH, DCH)):
                ncch = min(DCH, NCH - cb * DCH)
                xin = dn.tile([FIN, DCH * 128], BF16, tag="xin")
                nc.sync.dma_start(xin[:, 0:ncch * 128],
                                  xvT[:, cb * DCH * 128:(cb * DCH + ncch) * 128])
                rows = dn.tile([128, DCH * ROW1_U16], U16, tag="rows")
                rv16 = rows.bitcast(BF16).rearrange("p (c w) -> p c w", w=ROW1_U16)
                rv32 = rows.bitcast(F32).rearrange("p (c w) -> p c w", w=ROW1_U16 // 2)
                for b0 in range(0, ncch, 7):
                    nbc = min(7, ncch - b0)
                    ps_d = ps.tile([128, 7 * 69], F32, name="ps_d", tag="psH")
                    pv = ps_d.rearrange("p (c w) -> p c w", w=69)
                    for c in range(b0, b0 + nbc):
                        nc.tensor.matmul(out=pv[:, c - b0, :],
                                         lhsT=xin[:, c * 128:(c + 1) * 128],
                                         rhs=wall_sb[:, :], start=True, stop=True)
                    nc.scalar.activation(rv16[:, b0:b0 + nbc, 0:65],
                                         pv[:, 0:nbc, 0:65], AF.Copy)
                    nc.vector.tensor_copy(rv32[:, b0:b0 + nbc, ROW1_F32O:ROW1_F32O + 4],
                                          pv[:, 0:nbc, 65:69])
                nc.gpsimd.dma_start(
                    table[cb * DCH * 128:(cb * DCH + ncch) * 128, :]
                    .rearrange("(p c) w -> p c w", c=ncch),
                    rows.rearrange("p (c w) -> p c w", w=ROW1_U16)[:, 0:ncch, :])

            # patch the two zero rows (slot 0 of dev 0 and dev 4)
            zr = sb.tile([1, ROW1_U16], U16, tag="zr")
            nc.vector.memset(zr[:, :], 0)
            nc.vector.memset(zr.bitcast(BF16)[:, 64:65], PCOL_NEG)
            nc.sync.dma_start(table[0:1, :], zr[:, :])
            nc.sync.dma_start(table[halfrows:halfrows + 1, :], zr[:, :])

            # ---------------- xr rows (own slots only) ----------------
            NXCH = NSP // 128
            for cb in range(_ceil(NXCH, DCH)):
                nxch = min(DCH, NXCH - cb * DCH)
                xin = dn.tile([FIN, DCH * 128], BF16, tag="xrin")
                nc.sync.dma_start(xin[:, 0:nxch * 128],
                                  xrT[:, cb * DCH * 128:(cb * DCH + nxch) * 128])
                xrr = dn.tile([128, DCH * XR1_U16], U16, tag="xrr")
                xv16 = xrr.bitcast(BF16).rearrange("p (c w) -> p c w", w=XR1_U16)
                for b0 in range(0, nxch, 7):
                    nbc = min(7, nxch - b0)
                    ps_x = ps.tile([128, 7 * 65], F32, name="ps_x", tag="psH")
                    pxv = ps_x.rearrange("p (c w) -> p c w", w=65)
                    for c in range(b0, b0 + nbc):
                        nc.tensor.matmul(out=pxv[:, c - b0, :],
                                         lhsT=xin[:, c * 128:(c + 1) * 128],
                                         rhs=wxr_sb[:, :], start=True, stop=True)
                    if (b0 // 7) % 2 == 0:
                        nc.scalar.activation(xv16[:, b0:b0 + nbc, 0:65],
                                             pxv[:, 0:nbc, :], AF.Copy)
                    else:
                        nc.vector.tensor_copy(xv16[:, b0:b0 + nbc, 0:65],
                                              pxv[:, 0:nbc, :])
                nc.gpsimd.dma_start(
                    xr_dram[cb * DCH * 128:(cb * DCH + nxch) * 128, :]
                    .rearrange("(p c) w -> p c w", c=nxch),
                    xrr.rearrange("p (c w) -> p c w", w=XR1_U16)[:, 0:nxch, :])

            # ---------------- edge pass ----------------
            def edge_pass(lay, on_flush=None):
                if lay == 1:
                    ROWW, F32O, XRW = ROW1_U16, ROW1_F32O, XR1_U16
                    tabT, xrTd, Xs = table, xr_dram, X1
                    NP_, NN_, NF, NVAL = npos1, nneg1, 64, 4
                else:
                    ROWW, F32O, XRW = ROW2_U16, ROW2_F32O, XR2_U16
                    tabT, xrTd, Xs = l2tab, xr2_dram, X2
                    NP_, NN_, NF, NVAL = npos2, nneg2, 2, 2
                NV = NVAL + 1
                NLOG = NF + 1
                BAT_L = BAT if lay == 1 else BAT2
                FT_L = PSW - BAT_L + 1
                offs = [0, 0]       # idx offsets (u16 cols/8) per side
                ohst_off = [0, 0]
                ohsc_off = [0, 0]
                # rows finalized once a B-side class completes = next B
                # class's slot0 (skips K=0 classes and trailing pad rows)
                bcl = [c for side, mem in blocks if side == 1
                       for c, _, _ in mem]
                bseen, border = set(), []
                for c in bcl:
                    if id(c) not in bseen:
                        bseen.add(id(c))
                        border.append(c)
                nextb_slot0 = {id(c): (border[i + 1]["slot0"]
                                       if i + 1 < len(border) else NSP)
                               for i, c in enumerate(border)}
                st = {}             # per-side window state
                for side, members in blocks:
                    idx_sb = idxA_sb if side == 0 else idxB_sb
                    ohst_sb = ohstA_sb if side == 0 else ohstB_sb
                    ohsc_sb = ohscA_sb if side == 0 else ohscB_sb
                    ntot = sum(mm[2] for mm in members)

                    # ONE gather for the whole super (spans class boundaries)
                    STt = sb2.tile([128, SBW * ROWW], U16, name="STt", tag="ST")
                    c16 = offs[side] // 16
                    base = 0 if side == 0 else halfrows
                    nc.gpsimd.dma_gather(
                        out_ap=STt[:, 0:ntot * ROWW]
                        .rearrange("p (k w) -> p k w", w=ROWW),
                        in_ap=tabT[base:base + halfrows, :],
                        idxs_ap=idx_sb[:, c16:c16 + 8 * ntot],
                        num_idxs=128 * ntot, num_idxs_reg=128 * ntot,
                        elem_size=ROWW, single_packet=False)
                    offs[side] += 128 * ntot
                    Sb = STt.bitcast(BF16).rearrange("p (k w) -> p k w", w=ROWW)
                    Sf = STt.bitcast(F32).rearrange("p (k w) -> p k w", w=ROWW // 2)

                    moff = 0
                    for c, t0, nch in members:
                        K = c["KA"] if side == 0 else c["KB"]
                        m = c["mA"] if side == 0 else c["mB"]
                        te = c["teA"] if side == 0 else c["teB"]
                        tiles = c["tilesA"] if side == 0 else c["tilesB"]
                        slot0 = c["slot0"]
                        if t0 == 0:
                            st[side] = dict(fl0=0, psS=None)

                        # xr rows for this member's tiles
                        xrst = sb2.tile([128, SBW * XR1_U16], U16,
                                        name="xrst", tag="xrst")
                        xru = xrst.rearrange("p (k w) -> p k w", w=XR1_U16)
                        r0 = slot0 + t0 * m
                        nc.sync.dma_start(
                            xru[0:m, 0:nch, 0:XRW],
                            xrTd[r0:r0 + nch * m, 0:XRW]
                            .rearrange("(c g) w -> g c w", g=m))
                        xrb = xrst.bitcast(BF16).rearrange(
                            "p (k w) -> p k w", w=XR1_U16)

                        for b in range(_ceil(nch, BAT_L)):
                            nb = min(BAT_L, nch - b * BAT_L)
                            bsS = slice(moff + b * BAT_L, moff + b * BAT_L + nb)
                            bsx = slice(b * BAT_L, b * BAT_L + nb)
                            psH = ps.tile([128, BAT_L * NLOG], F32,
                                          name="psH", tag="psH")
                            pHv = psH.rearrange("p (b w) -> p b w", w=NLOG)
                            nc.tensor.matmul(
                                out=pHv[0:te, 0:nb, :],
                                lhsT=eye_sb[0:te, 0:te],
                                rhs=Sb[0:te, bsS, 0:NLOG],
                                start=True, stop=False)
                            nc.tensor.matmul(
                                out=pHv[0:te, 0:nb, :],
                                lhsT=ohst_sb[0:m, ohst_off[side]:ohst_off[side] + te],
                                rhs=xrb[0:m, bsx, 0:NLOG],
                                start=False, stop=True)
                            Hr = sb.tile([128, BAT_L * NF], BF16,
                                         name="Hr", tag="Hr")
                            Hv = Hr.rearrange("p (b w) -> p b w", w=NF)
                            nc.scalar.activation(
                                Hv[0:te, 0:nb, :], pHv[0:te, 0:nb, 0:NF], AF.Relu)
                            dt = sb.tile([128, BAT_L], F32, name="dt", tag="dt")
                            if NP_ > 0 and NN_ > 0:
                                Ap = sb.tile([128, BAT_L], F32,
                                             name="Ap", tag="Ap")
                                An = sb.tile([128, BAT_L], F32,
                                             name="An", tag="An")
                                nc.vector.tensor_reduce(
                                    out=Ap[0:te, 0:nb], in_=Hv[0:te, 0:nb, 0:NP_],
                                    axis=AX.X, op=ALU.add)
                                nc.vector.tensor_reduce(
                                    out=An[0:te, 0:nb], in_=Hv[0:te, 0:nb, NP_:NF],
                                    axis=AX.X, op=ALU.add)
                                nc.vector.tensor_tensor(
                                    out=dt[0:te, 0:nb], in0=Ap[0:te, 0:nb],
                                    in1=An[0:te, 0:nb], op=ALU.subtract)
                            else:
                                nc.vector.tensor_reduce(
                                    out=dt[0:te, 0:nb], in_=Hv[0:te, 0:nb, 0:NF],
                                    axis=AX.X, op=ALU.add)
                                if NN_ > 0:
                                    nc.vector.tensor_scalar(
                                        out=dt[0:te, 0:nb], in0=dt[0:te, 0:nb],
                                        scalar1=-1.0, scalar2=None, op0=ALU.mult)
                            ep = sb.tile([128, BAT_L], F32, name="ep", tag="ep")
                            nc.vector.tensor_tensor(
                                out=ep[0:te, 0:nb], in0=dt[0:te, 0:nb],
                                in1=pHv[0:te, 0:nb, NF], op=ALU.add)
                            # sval = [s * vals, s]; exp lands in the s col
                            sval = sb.tile([128, BAT_L * NV], F32,
                                           name="sv", tag="sv")
                            svt = sval.rearrange("p (b v) -> p b v", v=NV)
                            nc.scalar.activation(
                                svt[0:te, 0:nb, NV - 1], ep[0:te, 0:nb], AF.Exp,
                                scale=0.8)
                            nc.vector.tensor_tensor(
                                out=svt[0:te, 0:nb, 0:NVAL],
                                in0=Sf[0:te, bsS, F32O:F32O + NVAL],
                                in1=svt[0:te, 0:nb, NV - 1:NV]
                                .to_broadcast([te, nb, NVAL]),
                                op=ALU.mult)

                            # batched scatter into this side's psum window
                            jj = t0 + b * BAT_L - st[side]["fl0"]
                            if jj == 0:
                                st[side]["psS"] = psSp.tile(
                                    [128, PSW * NV], F32, name="psS", tag="psS")
                            psS = st[side]["psS"]
                            nc.tensor.matmul(
                                out=psS[0:m, jj * NV:(jj + nb) * NV],
                                lhsT=ohsc_sb[0:te, ohsc_off[side]:ohsc_off[side] + m],
                                rhs=sval[0:te, 0:nb * NV],
                                start=True, stop=True)

                            tg_last = t0 + b * BAT_L + nb - 1
                            if jj + nb >= FT_L or tg_last == tiles - 1:
                                flush(Xs[side], NV, m, slot0, st[side]["fl0"],
                                      jj + nb, psS)
                                st[side]["fl0"] = tg_last + 1
                                st[side]["psS"] = None
                                if on_flush is not None and side == 1:
                                    if tg_last == tiles - 1:
                                        done = nextb_slot0.get(id(c), NSP)
                                    else:
                                        done = slot0 + (tg_last + 1) * m
                                    on_flush(done)

                        moff += nch
                        if t0 + nch >= tiles:  # class-side done: advance consts
                            ohst_off[side] += te
                            ohsc_off[side] += m

            def flush(Xsd, NV, m, slot0, f_t0, ntl, psS):
                # PSUM -> SBUF -> DRAM accumulator (node-slot order)
                sc = sb.tile([128, PSW * NV], F32, name="sc", tag="sc")
                nc.vector.tensor_copy(sc[0:m, 0:ntl * NV], psS[0:m, 0:ntl * NV])
                r0 = slot0 + f_t0 * m
                nc.sync.dma_start(
                    Xsd[r0:r0 + ntl * m, :].rearrange("(j p) v -> p j v", p=m),
                    sc.rearrange("p (j v) -> p j v", v=NV)[0:m, 0:ntl, :])

            # ------------- final pass: combine sides, normalize -------------
            def final_chunk(lay, cb):
                Xs = X1 if lay == 1 else X2
                NV = 5 if lay == 1 else 3
                if True:
                    nfc = min(FCH, NXCH - cb * FCH)
                    r0 = cb * FCH * 128
                    ta = sb.tile([128, FCH * NV], F32, tag="fta")
                    tb = sb.tile([128, FCH * NV], F32, tag="ftb")
                    tav = ta.rearrange("p (c v) -> p c v", v=NV)
                    tbv = tb.rearrange("p (c v) -> p c v", v=NV)
                    for s, tv in ((0, tav), (1, tbv)):
                        nc.sync.dma_start(
                            tv[:, 0:nfc, :],
                            Xs[s][r0:r0 + nfc * 128, :]
                            .rearrange("(c p) v -> p c v", p=128))
                    tt = sb.tile([128, FCH * NV], F32, tag="ftt")
                    ttv = tt.rearrange("p (c v) -> p c v", v=NV)
                    nc.vector.tensor_tensor(
                        out=ttv[:, 0:nfc, :], in0=tav[:, 0:nfc, :],
                        in1=tbv[:, 0:nfc, :], op=ALU.add)
                    rcpi = sb.tile([128, FCH], F32, tag="rcpi")
                    nc.vector.tensor_scalar(
                        out=rcpi[:, 0:nfc], in0=ttv[:, 0:nfc, NV - 1],
                        scalar1=1e-30, scalar2=None, op0=ALU.add)
                    rcp = sb.tile([128, FCH], F32, tag="rcp")
                    nc.vector.reciprocal(rcp[:, 0:nfc], rcpi[:, 0:nfc])
                    scv = sb.tile([128, FCH * 4], F32, tag="fsc")
                    sv = scv.rearrange("p (c v) -> p c v", v=4)
                    nc.vector.tensor_tensor(
                        out=sv[:, 0:nfc, 0:NV - 1], in0=ttv[:, 0:nfc, 0:NV - 1],
                        in1=rcp[:, 0:nfc].rearrange("p (c o) -> p c o", o=1)
                        .to_broadcast([128, nfc, NV - 1]),
                        op=ALU.mult)
                    if lay == 1:
                        l2r = sb.tile([128, FCH * ROW2_USED], U16, tag="l2r")
                        lb = l2r.bitcast(BF16).rearrange("p (j v) -> p j v", v=ROW2_USED)
                        lf = l2r.bitcast(F32).rearrange("p (j v) -> p j v", v=ROW2_USED // 2)
                        x2r = sb.tile([128, FCH * XR2_U16], U16, tag="x2r")
                        xb2 = x2r.bitcast(BF16).rearrange("p (j v) -> p j v", v=XR2_U16)
                        t1 = sb.tile([128, FCH], F32, tag="ft1")
                        t2 = sb.tile([128, FCH], F32, tag="ft2")
                        for cc in range(2):
                            nc.vector.tensor_scalar(
                                out=lb[:, 0:nfc, cc], in0=sv[:, 0:nfc, perm2[cc]],
                                scalar1=float(a2p[cc]),
                                scalar2=float(a2p[cc] * cy[perm2[cc]]),
                                op0=ALU.mult, op1=ALU.add)
                            nc.vector.tensor_scalar(
                                out=xb2[:, 0:nfc, cc], in0=sv[:, 0:nfc, 2 + perm2[cc]],
                                scalar1=float(a2p[cc]),
                                scalar2=float(a2p[cc] * cz[perm2[cc]]),
                                op0=ALU.mult, op1=ALU.add)
                        nc.vector.tensor_scalar(
                            out=t1[:, 0:nfc], in0=sv[:, 0:nfc, 0],
                            scalar1=float(0.25 * att2[0]),
                            scalar2=float(0.25 * (att2 @ cy)),
                            op0=ALU.mult, op1=ALU.add)
                        nc.vector.tensor_scalar(
                            out=t2[:, 0:nfc], in0=sv[:, 0:nfc, 1],
                            scalar1=float(0.25 * att2[1]), scalar2=None, op0=ALU.mult)
                        nc.vector.tensor_tensor(
                            out=lb[:, 0:nfc, 2], in0=t1[:, 0:nfc],
                            in1=t2[:, 0:nfc], op=ALU.add)
                        nc.vector.tensor_scalar(
                            out=t1[:, 0:nfc], in0=sv[:, 0:nfc, 2],
                            scalar1=float(0.25 * att2[0]),
                            scalar2=float(0.25 * (att2 @ cz)),
                            op0=ALU.mult, op1=ALU.add)
                        nc.vector.tensor_scalar(
                            out=t2[:, 0:nfc], in0=sv[:, 0:nfc, 3],
                            scalar1=float(0.25 * att2[1]), scalar2=None, op0=ALU.mult)
                        nc.vector.tensor_tensor(
                            out=xb2[:, 0:nfc, 2], in0=t1[:, 0:nfc],
                            in1=t2[:, 0:nfc], op=ALU.add)
                        nc.vector.memset(xb2[:, 0:nfc, 3], 0.0)
                        for cc in range(2):
                            nc.vector.tensor_scalar(
                                out=lf[:, 0:nfc, ROW2_F32O + cc],
                                in0=sv[:, 0:nfc, cc],
                                scalar1=float(cy[cc]), scalar2=None, op0=ALU.add)
                        nc.sync.dma_start(
                            l2slice[r0:r0 + nfc * 128, :]
                            .rearrange("(j p) w -> p j w", p=128),
                            l2r.rearrange("p (j v) -> p j v", v=ROW2_USED)[
                                :, 0:nfc, :])
                        nc.sync.dma_start(
                            xr2_dram[r0:r0 + nfc * 128, 0:XR2_U16]
                            .rearrange("(j p) w -> p j w", p=128),
                            x2r.rearrange("p (j v) -> p j v", v=XR2_U16)[:, 0:nfc, :])
                        for i, (fire_k, p0, p1) in enumerate(_pieces):
                            if cb == fire_k:
                                ag_piece(i, p0, p1)
                    else:
                        o2 = sb.tile([128, FCH * 2], F32, tag="o2")
                        o2v = o2.rearrange("p (j v) -> p j v", v=2)
                        for cc in range(2):
                            nc.vector.tensor_scalar(
                                out=o2v[:, 0:nfc, cc], in0=sv[:, 0:nfc, cc],
                                scalar1=float(b2[cc]), scalar2=None, op0=ALU.add)
                        nc.sync.dma_start(
                            out2[r0:r0 + nfc * 128, :].rearrange("(j p) w -> p j w", p=128),
                            o2v[:, 0:nfc, :])

            # chunked AllGather: each node-range piece fires as soon as its
            # final-pass chunk lands, overlapping the collective + table
            # expansion with the layer-1 tail
            def ag_piece(i, r0, r1):
                if i == 0:
                    # zero row 0 of l2slice (att cols 0, p2 col -100, vals 0)
                    zr2 = sb.tile([1, ROW2_USED], U16, name="zr2", tag="zr2")
                    nc.vector.memset(zr2[:, :], 0)
                    nc.vector.memset(zr2.bitcast(BF16)[:, 2:3], PCOL_NEG)
                    nc.sync.dma_start(l2slice[0:1, :], zr2[:, :])
                nrows = r1 - r0
                gv = l2gathP[i].rearrange("(d r) w -> d r w", r=nrows)
                nc.gpsimd.collective_compute(
                    "AllGather", ALU.bypass,
                    replica_groups=[list(range(NCORE))],
                    ins=[l2slice[r0:r1, :]],
                    outs=[l2gathP[i][:, :]],
                )
                tv = l2tab.rearrange("(d r) w -> d r w", r=NSP)
                nc.sync.dma_start(tv[:, r0:r1, 0:ROW2_USED],
                                  gv[:, :, :])

            def mk_on_flush(lay, fin):
                def on_flush(rows_done):
                    while fin["nxt"] < _nfin:
                        k = fin["nxt"]
                        nfc = min(FCH, NXCH - k * FCH)
                        if (k * FCH + nfc) * 128 > rows_done:
                            break
                        final_chunk(lay, k)
                        fin["nxt"] += 1
                return on_flush

            fin1 = dict(nxt=0)
            edge_pass(1, mk_on_flush(1, fin1))
            mk_on_flush(1, fin1)(NSP)  # tail
            fin2 = dict(nxt=0)
            edge_pass(2, mk_on_flush(2, fin2))
            mk_on_flush(2, fin2)(NSP)  # tail

    return nc, NSP


def run_device(meta, W, x, trace=False):
    nc, NSP = build_program(meta, W)
    NS = meta["NS"]
    classes = meta["classes"]
    bf = ml_dtypes.bfloat16
    ohstA, ohstB, ohscA, ohscB = make_oh_consts(classes)

    xs_all = np.concatenate(
        [meta["slot_x"](d, x) for d in range(NCORE)], axis=0).astype(bf)
    xvT_np = np.ascontiguousarray(xs_all[dense_colmap(len(xs_all))].T)

    in_maps = []
    for d in range(NCORE):
        xsd = meta["slot_x"](d, x).astype(bf)
        im = dict(
            xvT=xvT_np,
            xrT=np.ascontiguousarray(xsd[dense_colmap(len(xsd))].T),
            wall=np.asarray(W["wall"]),
            wxr=np.asarray(W["wxr"]),
            idxA=meta["idxA"][d],
            idxB=meta["idxB"][d],
            eye=np.eye(128, dtype=bf),
            ohstA=ohstA, ohstB=ohstB, ohscA=ohscA, ohscB=ohscB,
        )
        in_maps.append(im)

    if not nc.is_finalized():
        nc.finalize()
    res = run_bass_kernel_spmd(nc, in_maps, list(range(NCORE)), trace=trace)
    outs = res.results
    out = np.zeros((N, FOUT), dtype=np.float32)
    for d in range(NCORE):
        o = outs[d]["out2"]
        valid = meta["node_at"][d] >= 0
        out[meta["node_at"][d][valid]] = o[:NSP][valid]
    return out, res


# --------------------------------------------------------------------------
# entry
# --------------------------------------------------------------------------

def kernel(**inputs):
    x = np.asarray(inputs["x"], dtype=np.float32)
    meta = build_schedule(np.asarray(inputs["edge_index"]))
    W = prep_weights(
        np.asarray(inputs["Wl1"], np.float32), np.asarray(inputs["Wr1"], np.float32),
        np.asarray(inputs["att1"], np.float32), np.asarray(inputs["b1"], np.float32),
        np.asarray(inputs["Wl2"], np.float32), np.asarray(inputs["Wr2"], np.float32),
        np.asarray(inputs["att2"], np.float32), np.asarray(inputs["b2"], np.float32),
    )
    if os.environ.get("GAT_MOCK"):
        return mock_run(meta, W, x)
    out, _res = run_device(meta, W, x)
    return out


if __name__ == "__main__":
    pass
